# revision 6
# baseline (speedup 1.0000x reference)
"""FLASH (gated-attention-unit / FLASH_ShareA_FFConvM) block on 8 trn2 cores.

Sharding: core c -> (sample b = c//4, seq chunk j = c%4, 2048 tokens each).
Halo regions are recomputed locally per core; the only collectives are the
linear-attention lkv/lku AllReduce and the GroupNorm stats AllReduce, both
within the 4-core sample groups [[0..3],[4..7]].

Self-contained: hardcodes all shapes; no file reads.
"""
import numpy as np
import concourse.bass as bass
import concourse.mybir as mybir
import concourse.tile as tile
from concourse import bacc
from concourse.bass_utils import run_bass_kernel_spmd
from concourse.masks import make_identity

F32 = mybir.dt.float32
F16 = mybir.dt.float16
F32R = mybir.dt.float32r
Alu = mybir.AluOpType
Act = mybir.ActivationFunctionType

B, D, S, H, QK, GSZ, KS = 2, 512, 8192, 2048, 128, 256, 17
NL = 2
TOK = 2048
HXPAD = 640
HXLEN = TOK + 2 * HXPAD      # 3328
H1LEN = 2800                 # h1 scratch = rel [-384, 2416)

# layer grids: grid idx p in [0, ntt*128), rel token = base + p.
# src (hx / h1) index = grid + 128 for both layers by construction.
LCFG = [
    dict(base=-512, ntt=24, att_lo=0, att_hi=22, quarters=(768, 768, 768, 768),
         own_lo=4, own_hi=19, out_g0=128, out_len=2800, toff=0),
    dict(base=-256, ntt=20, att_lo=1, att_hi=18, quarters=(768, 768, 768, 256),
         own_lo=2, own_hi=17, out_g0=256, out_len=2048, toff=256),
]
NTTM = 24
RG = [[0, 1, 2, 3], [4, 5, 6, 7]]
DEBUG_OUT = True

_CACHE = {}


def _pT(a):
    """Swap the first two AP dims (DRAM<->SBUF dim-order matching)."""
    return bass.AP(tensor=a.tensor, offset=a.offset,
                   ap=[a.ap[1], a.ap[0]] + list(a.ap[2:]))


def _b3(t, w, n):
    """View [128, W] tile slice [0:w] as [128, n, w] with stride-0 mid dim."""
    a = t[:, 0:w]
    return bass.AP(tensor=a.tensor, offset=a.offset,
                   ap=[a.ap[0], [0, n], [1, w]])


def build_nc():
    nc = bacc.Bacc("TRN2", target_bir_lowering=False, debug=False, num_devices=8)

    hx = nc.dram_tensor("hx", [D, HXLEN], F32, kind="ExternalInput")
    cosd = nc.dram_tensor("cosd", [32, 3072], F16, kind="ExternalInput")
    sind = nc.dram_tensor("sind", [32, 3072], F16, kind="ExternalInput")
    permT = nc.dram_tensor("permT", [32, 32], F32, kind="ExternalInput")
    wh_in = nc.dram_tensor("wh_in", [NL, D, H], F32, kind="ExternalInput")
    wq_in = nc.dram_tensor("wq_in", [NL, D, QK], F32, kind="ExternalInput")
    wo_in = nc.dram_tensor("wo_in", [NL, 2 * D, D], F16, kind="ExternalInput")
    kh_in = nc.dram_tensor("kh_in", [NL, 128, 16, KS], F32, kind="ExternalInput")
    kq_in = nc.dram_tensor("kq_in", [NL, 128, 1, KS], F32, kind="ExternalInput")
    ko_in = nc.dram_tensor("ko_in", [NL, 128, 4, KS], F32, kind="ExternalInput")
    bh_in = nc.dram_tensor("bh_in", [NL, 128, 16], F32, kind="ExternalInput")
    bq_in = nc.dram_tensor("bq_in", [NL, 128, 1], F32, kind="ExternalInput")
    bo_in = nc.dram_tensor("bo_in", [NL, 128, 4], F32, kind="ExternalInput")
    gam_in = nc.dram_tensor("gam_in", [NL, 128, 4], F32, kind="ExternalInput")
    bet_in = nc.dram_tensor("bet_in", [NL, 128, 4], F32, kind="ExternalInput")
    ln_in = nc.dram_tensor("ln_in", [128, 4, 2], F32, kind="ExternalInput")
    gn_in2 = nc.dram_tensor("gn_in2", [128, 4, 2], F32, kind="ExternalInput")

    y_out = nc.dram_tensor("y_out", [D, TOK], F32, kind="ExternalOutput")
    if DEBUG_OUT:
        h1dbg = nc.dram_tensor("h1dbg", [128, 4, H1LEN], F32,
                               kind="ExternalOutput")

    hx_r = hx.rearrange("(a p) t -> p a t", p=128)
    y_r = y_out.rearrange("(a p) t -> p a t", p=128)
    wh_r = wh_in.rearrange("l (k p) h -> l p k h", p=128)
    wq_r = wq_in.rearrange("l (k p) h -> l p k h", p=128)
    wo_r = wo_in.rearrange("l (k p) h -> l p k h", p=128)

    with tile.TileContext(nc) as tc:
        with tc.tile_pool(name="const", bufs=1) as cpool, \
             tc.tile_pool(name="dram", bufs=1, space="DRAM") as dpool:
            ident = cpool.tile([128, 128], F16)
            make_identity(nc, ident)
            perm = cpool.tile([32, 32], F16)
            nc.gpsimd.dma_start(out=perm, in_=permT[:])
            cos_t = cpool.tile([32, 3072], F16)
            nc.sync.dma_start(out=cos_t, in_=cosd[:])
            sin_t = cpool.tile([32, 3072], F16)
            nc.sync.dma_start(out=sin_t, in_=sind[:])
            ones_f = cpool.tile([128, 1], F32)
            nc.vector.memset(ones_f, 1.0)
            ones_r = cpool.tile([128, 1], F32R)
            nc.vector.tensor_copy(ones_r, ones_f)
            lnp = cpool.tile([128, 4, 2], F32)
            nc.sync.dma_start(out=lnp, in_=ln_in[:])
            gnp = cpool.tile([128, 4, 2], F32)
            nc.sync.dma_start(out=gnp, in_=gn_in2[:])

            v_dram = dpool.tile([NTTM, 128, 1024], F16, tag="v_dram")
            u_dram = dpool.tile([NTTM, 128, 1024], F16, tag="u_dram")
            cc_in = dpool.tile([128, 2048], F16, tag="cc_in")
            cc_out = dpool.tile([128, 2048], F16, tag="cc_out")
            gcc_in = dpool.tile([1, 256], F32, tag="gcc_in")
            gcc_out = dpool.tile([1, 256], F32, tag="gcc_out")
            h1d = dpool.tile([128, 4, H1LEN], F32, tag="h1d")
            h2d = dpool.tile([128, 4, TOK], F32, tag="h2d")

            for l in range(NL):
                cfg = LCFG[l]
                ntt = cfg["ntt"]
                NT = ntt * 128
                toff = cfg["toff"]
                src_r = hx_r if l == 0 else h1d

                with tc.tile_pool(name="lay", bufs=1) as lp:
                    wh_t = lp.tile([128, 4, H], F32R, tag="wh_t")
                    nc.gpsimd.dma_start(out=wh_t, in_=wh_r[l])
                    wq_t = lp.tile([128, 4, QK], F32R, tag="wq_t")
                    nc.gpsimd.dma_start(out=wq_t, in_=wq_r[l])
                    wo_t = lp.tile([128, 8, D], F16, tag="wo_t")
                    nc.sync.dma_start(out=wo_t, in_=wo_r[l])
                    kh_t = lp.tile([128, 16, KS], F32, tag="kh_t")
                    nc.sync.dma_start(out=kh_t, in_=kh_in[l])
                    kq_t = lp.tile([128, 1, KS], F32, tag="kq_t")
                    nc.sync.dma_start(out=kq_t, in_=kq_in[l])
                    ko_t = lp.tile([128, 4, KS], F32, tag="ko_t")
                    nc.sync.dma_start(out=ko_t, in_=ko_in[l])
                    bh_t = lp.tile([128, 16], F32, tag="bh_t")
                    nc.sync.dma_start(out=bh_t, in_=bh_in[l])
                    bq_t = lp.tile([128, 1], F32, tag="bq_t")
                    nc.sync.dma_start(out=bq_t, in_=bq_in[l])
                    bo_t = lp.tile([128, 4], F32, tag="bo_t")
                    nc.sync.dma_start(out=bo_t, in_=bo_in[l])
                    gam_t = lp.tile([128, 4], F32, tag="gam_t")
                    nc.sync.dma_start(out=gam_t, in_=gam_in[l])
                    bet_t = lp.tile([128, 4], F32, tag="bet_t")
                    nc.sync.dma_start(out=bet_t, in_=bet_in[l])

                    heads = lp.tile([128, 4, NTTM * 128], F16, tag="heads")
                    lk_tm = lp.tile([128, NTTM, 128], F16, tag="lk_tm")
                    lkv_h = lp.tile([128, 2048], F16, tag="lkv_h")

                    # ================= stage 1 =================
                    with tc.tile_pool(name="s1", bufs=1) as s1p, \
                         tc.tile_pool(name="s1d", bufs=2) as s1dp, \
                         tc.tile_pool(name="s1t", bufs=3) as s1tp, \
                         tc.tile_pool(name="pmm", bufs=3, space="PSUM") as pmm, \
                         tc.tile_pool(name="pnn", bufs=1, space="PSUM") as pnn, \
                         tc.tile_pool(name="ptr", bufs=2, space="PSUM") as ptrp, \
                         tc.tile_pool(name="prt", bufs=2, space="PSUM") as prtp:
                        qp = 0
                        for q, Qw in enumerate(cfg["quarters"]):
                            vh_q = s1p.tile([128, 16, 784], F16, tag="vh_q")
                            qh_q = s1p.tile([128, 784], F16, tag="qh_q")
                            nck = (Qw + 383) // 384
                            for ci in range(nck):
                                p0 = qp + 384 * ci
                                W = min(384, Qw - 384 * ci)
                                Wm = W + 16
                                a0 = p0 + 120
                                nx = s1dp.tile([128, 4, 400], F32, tag="nx")
                                nc.sync.dma_start(
                                    out=nx[:, 0:2, 0:Wm],
                                    in_=src_r[:, 0:2, a0 - 1:a0 - 1 + Wm])
                                nc.sync.dma_start(
                                    out=nx[:, 2:4, 0:Wm],
                                    in_=src_r[:, 2:4, a0:a0 + Wm])
                                xsq = s1dp.tile([128, 4, 400], F32R, tag="xsq", bufs=1)
                                nc.scalar.activation(xsq[:, :, 0:Wm],
                                                     nx[:, :, 0:Wm], Act.Square)
                                psn = pnn.tile([1, 400], F32, tag="psn")
                                for k in range(4):
                                    nc.tensor.matmul(psn[:, 0:Wm], ones_r,
                                                     xsq[:, k, 0:Wm],
                                                     start=(k == 0), stop=(k == 3))
                                nrm = s1dp.tile([1, 400], F32, tag="nrm")
                                nc.scalar.activation(nrm[:, 0:Wm], psn[:, 0:Wm],
                                                     Act.Sqrt, scale=1.0 / D)
                                nc.vector.tensor_scalar_max(nrm[:, 0:Wm],
                                                            nrm[:, 0:Wm], 1e-5)
                                inv = s1dp.tile([1, 400], F32, tag="inv")
                                nc.vector.reciprocal(inv[:, 0:Wm], nrm[:, 0:Wm])
                                invb = s1dp.tile([128, 400], F32, tag="invb", bufs=1)
                                nc.gpsimd.partition_broadcast(invb[:, 0:Wm],
                                                              inv[0:1, 0:Wm])
                                nxn = s1dp.tile([128, 4, 400], F32R, tag="nxn")
                                nc.vector.tensor_tensor(nxn[:, :, 0:Wm],
                                                        nx[:, :, 0:Wm],
                                                        _b3(invb, Wm, 4), Alu.mult)
                                wpos = p0 - qp
                                for m in range(16):
                                    psh = pmm.tile([128, 400], F32, tag="psh")
                                    for k in range(4):
                                        nc.tensor.matmul(
                                            psh[:, 0:Wm],
                                            wh_t[:, k, m * 128:(m + 1) * 128],
                                            nxn[:, k, 0:Wm],
                                            start=(k == 0), stop=(k == 3))
                                    nc.scalar.activation(
                                        vh_q[:, m, wpos:wpos + Wm], psh[:, 0:Wm],
                                        Act.Silu, bias=bh_t[:, m:m + 1])
                                psq = pmm.tile([128, 400], F32, tag="psh")
                                for k in range(4):
                                    nc.tensor.matmul(psq[:, 0:Wm], wq_t[:, k, :],
                                                     nxn[:, k, 0:Wm],
                                                     start=(k == 0), stop=(k == 3))
                                nc.scalar.activation(qh_q[:, wpos:wpos + Wm],
                                                     psq[:, 0:Wm], Act.Silu,
                                                     bias=bq_t[:, 0:1])
                            # depthwise conv (hidden) + transpose + store
                            ntq = Qw // 128
                            for m in range(16):
                                vst = s1dp.tile([128, 768], F16, tag="vst")
                                tmp = s1tp.tile([128, 768], F16, tag="tmp")
                                nc.vector.tensor_scalar_mul(
                                    tmp[:, 0:Qw], vh_q[:, m, 0:Qw], kh_t[:, m, 0:1])
                                nc.vector.tensor_tensor(
                                    vst[:, 0:Qw], vh_q[:, m, 8:8 + Qw],
                                    tmp[:, 0:Qw], Alu.add)
                                for j in range(1, KS):
                                    tmp = s1tp.tile([128, 768], F16, tag="tmp")
                                    nc.vector.tensor_scalar_mul(
                                        tmp[:, 0:Qw], vh_q[:, m, j:j + Qw],
                                        kh_t[:, m, j:j + 1])
                                    nc.vector.tensor_tensor(
                                        vst[:, 0:Qw], vst[:, 0:Qw], tmp[:, 0:Qw],
                                        Alu.add)
                                vd = v_dram if m < 8 else u_dram
                                ms = (m % 8) * 128
                                for b0 in range(0, ntq, 4):
                                    bn = min(4, ntq - b0)
                                    ptr = ptrp.tile([128, 4, 128], F16, tag="ptr")
                                    for bi in range(bn):
                                        tt = b0 + bi
                                        nc.tensor.transpose(
                                            ptr[:, bi, :],
                                            vst[:, tt * 128:(tt + 1) * 128], ident)
                                    stg = s1tp.tile([128, 4, 128], F16, tag="stg")
                                    nc.scalar.activation(stg[:, 0:bn, :],
                                                         ptr[:, 0:bn, :], Act.Copy)
                                    tt0 = qp // 128 + b0
                                    nc.sync.dma_start(
                                        out=_pT(vd[tt0:tt0 + bn, :, ms:ms + 128]),
                                        in_=stg[:, 0:bn, :])
                            # qk conv, heads affine, rotary, lk transpose
                            qst = s1p.tile([128, 768], F16, tag="qst")
                            tmq = s1tp.tile([128, 768], F16, tag="tmp")
                            nc.vector.tensor_scalar_mul(
                                tmq[:, 0:Qw], qh_q[:, 0:Qw], kq_t[:, 0, 0:1])
                            nc.vector.tensor_tensor(qst[:, 0:Qw], qh_q[:, 8:8 + Qw],
                                                    tmq[:, 0:Qw], Alu.add)
                            for j in range(1, KS):
                                tmq = s1tp.tile([128, 768], F16, tag="tmp")
                                nc.vector.tensor_scalar_mul(
                                    tmq[:, 0:Qw], qh_q[:, j:j + Qw],
                                    kq_t[:, 0, j:j + 1])
                                nc.vector.tensor_tensor(qst[:, 0:Qw], qst[:, 0:Qw],
                                                        tmq[:, 0:Qw], Alu.add)
                            for hh in range(4):
                                nc.vector.tensor_scalar(
                                    heads[:, hh, qp:qp + Qw], qst[:, 0:Qw],
                                    gam_t[:, hh:hh + 1], bet_t[:, hh:hh + 1],
                                    Alu.mult, Alu.add)
                            for hh in range(4):
                                for r0 in range(0, Qw, 384):
                                    rw = min(384, Qw - r0)
                                    g0 = qp + r0
                                    prot = prtp.tile([32, 384], F32, tag="prot")
                                    nc.tensor.matmul(prot[:, 0:rw], perm,
                                                     heads[0:32, hh, g0:g0 + rw],
                                                     start=True, stop=True)
                                    tco = s1tp.tile([32, 384], F16, tag="tco")
                                    nc.vector.tensor_tensor(
                                        tco[:, 0:rw], heads[0:32, hh, g0:g0 + rw],
                                        cos_t[:, toff + g0:toff + g0 + rw],
                                        Alu.mult)
                                    tsi = s1tp.tile([32, 384], F16, tag="tsi")
                                    nc.vector.tensor_tensor(
                                        tsi[:, 0:rw], prot[:, 0:rw],
                                        sin_t[:, toff + g0:toff + g0 + rw],
                                        Alu.mult)
                                    nc.vector.tensor_tensor(
                                        heads[0:32, hh, g0:g0 + rw], tco[:, 0:rw],
                                        tsi[:, 0:rw], Alu.add)
                            for b0 in range(0, ntq, 4):
                                bn = min(4, ntq - b0)
                                ptr = ptrp.tile([128, 4, 128], F16, tag="ptr")
                                for bi in range(bn):
                                    tt = qp // 128 + b0 + bi
                                    nc.tensor.transpose(
                                        ptr[:, bi, :],
                                        heads[:, 3, tt * 128:(tt + 1) * 128],
                                        ident)
                                tt0 = qp // 128 + b0
                                nc.scalar.activation(lk_tm[:, tt0:tt0 + bn, :],
                                                     ptr[:, 0:bn, :], Act.Copy)
                            qp += Qw

                    # ============ lkv / lku accumulate + AllReduce ============
                    with tc.tile_pool(name="pkv", bufs=1, space="PSUM") as pkvp, \
                         tc.tile_pool(name="s2", bufs=2) as s2p:
                        pkv = [pkvp.tile([128, 512], F32, tag=f"pkv{i}", name=f"pkv{i}")
                               for i in range(2)]
                        pku = [pkvp.tile([128, 512], F32, tag=f"pku{i}", name=f"pku{i}")
                               for i in range(2)]
                        o_lo, o_hi = cfg["own_lo"], cfg["own_hi"]
                        for tt in range(o_lo, o_hi + 1):
                            vv = s2p.tile([128, 1024], F16, tag="vv")
                            nc.sync.dma_start(out=vv, in_=v_dram[tt])
                            uu = s2p.tile([128, 1024], F16, tag="uu")
                            nc.sync.dma_start(out=uu, in_=u_dram[tt])
                            st = (tt == o_lo)
                            sp = (tt == o_hi)
                            for i in range(2):
                                nc.tensor.matmul(pkv[i], lk_tm[:, tt, :],
                                                 vv[:, i * 512:(i + 1) * 512],
                                                 start=st, stop=sp)
                                nc.tensor.matmul(pku[i], lk_tm[:, tt, :],
                                                 uu[:, i * 512:(i + 1) * 512],
                                                 start=st, stop=sp)
                        kvs = s2p.tile([128, 2048], F16, tag="kvs")
                        for i in range(2):
                            nc.scalar.activation(kvs[:, i * 512:(i + 1) * 512],
                                                 pkv[i], Act.Copy, scale=1.0 / S)
                            nc.scalar.activation(
                                kvs[:, 1024 + i * 512:1024 + (i + 1) * 512],
                                pku[i], Act.Copy, scale=1.0 / S)
                        nc.sync.dma_start(out=cc_in[:], in_=kvs)
                        nc.gpsimd.collective_compute(
                            "AllReduce", Alu.add, replica_groups=RG,
                            ins=[cc_in.opt()], outs=[cc_out.opt()])
                        nc.sync.dma_start(out=lkv_h[:], in_=cc_out[:])

                    # ============ stage 2a: scores -> A for all groups ========
                    atp_cm = tc.tile_pool(name="att", bufs=1)
                    atp = atp_cm.__enter__()
                    oh_all = atp.tile([128, 4, NTTM * 128], F16, tag="oh_all")
                    a_st = atp.tile([128, 12, 2, 256], F16, tag="a_st")
                    with tc.tile_pool(name="pss", bufs=3, space="PSUM") as pssp, \
                         tc.tile_pool(name="s3", bufs=3) as s3p:
                        for g in range(ntt // 2):
                            gi = g * 256
                            for jt in range(2):
                                pss = pssp.tile([128, 256], F32, tag="pss")
                                nc.tensor.matmul(
                                    pss,
                                    heads[:, 2, gi + jt * 128:gi + (jt + 1) * 128],
                                    heads[:, 0, gi:gi + 256],
                                    start=True, stop=True)
                                rl = s3p.tile([128, 256], F16, tag="rl")
                                nc.scalar.activation(rl, pss, Act.Relu,
                                                     scale=1.0 / GSZ)
                                nc.vector.tensor_tensor(a_st[:, g, jt, :], rl, rl,
                                                        Alu.mult)

                    # ============ stage 2b: attention + gating + out mm =======
                    with tc.tile_pool(name="pav", bufs=1, space="PSUM") as pavp, \
                         tc.tile_pool(name="png", bufs=2, space="PSUM") as pngp, \
                         tc.tile_pool(name="poh", bufs=2, space="PSUM") as pohp, \
                         tc.tile_pool(name="s4", bufs=2) as s4p:
                        for g in range(ntt // 2):
                            its = [t for t in (2 * g, 2 * g + 1)
                                   if cfg["att_lo"] <= t <= cfg["att_hi"]]
                            if not its:
                                continue
                            vvg = s4p.tile([128, 2, 1024], F16, tag="vvg")
                            nc.sync.dma_start(out=vvg,
                                              in_=_pT(v_dram[2 * g:2 * g + 2]))
                            uug = s4p.tile([128, 2, 1024], F16, tag="uug")
                            nc.sync.dma_start(out=uug,
                                              in_=_pT(u_dram[2 * g:2 * g + 2]))
                            for it in its:
                                il = it - 2 * g
                                pav = pavp.tile([128, 1024], F32, tag="pav")
                                pau = pavp.tile([128, 1024], F32, tag="pau")
                                for eh in range(2):
                                    es = eh * 512
                                    for jt in range(2):
                                        nc.tensor.matmul(
                                            pav[:, es:es + 512],
                                            a_st[:, g, jt,
                                                 il * 128:(il + 1) * 128],
                                            vvg[:, jt, es:es + 512],
                                            start=(jt == 0), stop=False)
                                        nc.tensor.matmul(
                                            pau[:, es:es + 512],
                                            a_st[:, g, jt,
                                                 il * 128:(il + 1) * 128],
                                            uug[:, jt, es:es + 512],
                                            start=(jt == 0), stop=False)
                                    nc.tensor.matmul(
                                        pav[:, es:es + 512],
                                        heads[:, 1, it * 128:(it + 1) * 128],
                                        lkv_h[:, es:es + 512],
                                        start=False, stop=True)
                                    nc.tensor.matmul(
                                        pau[:, es:es + 512],
                                        heads[:, 1, it * 128:(it + 1) * 128],
                                        lkv_h[:, 1024 + es:1024 + es + 512],
                                        start=False, stop=True)
                                s1g = s4p.tile([128, 1024], F16, tag="s1g")
                                nc.vector.tensor_tensor(s1g, pav, uug[:, il, :],
                                                        Alu.mult)
                                sgg = s4p.tile([128, 1024], F16, tag="sgg")
                                nc.scalar.activation(sgg, s1g, Act.Sigmoid)
                                s2g = s4p.tile([128, 1024], F16, tag="s2g")
                                nc.vector.tensor_tensor(s2g, pau, vvg[:, il, :],
                                                        Alu.mult)
                                gt = s4p.tile([128, 1024], F16, tag="gt")
                                nc.vector.tensor_tensor(gt, s2g, sgg, Alu.mult)
                                gsq = s4p.tile([128, 1024], F16, tag="gsq")
                                nacc = s4p.tile([128, 1], F32, tag="nacc")
                                nc.scalar.activation(gsq, gt, Act.Square,
                                                     accum_out=nacc)
                                nn = s4p.tile([128, 1], F32, tag="nn")
                                nc.scalar.activation(nn, nacc, Act.Sqrt,
                                                     scale=1.0 / 1024)
                                nc.vector.tensor_scalar_max(nn, nn, 1e-5)
                                ninv = s4p.tile([128, 1], F32, tag="ninv")
                                nc.vector.reciprocal(ninv, nn)
                                ng = s4p.tile([128, 1024], F16, tag="ng")
                                nc.vector.tensor_scalar_mul(ng, gt, ninv)
                                ngT = s4p.tile([128, 8, 128], F16, tag="ngT")
                                for b0 in range(0, 8, 4):
                                    png = pngp.tile([128, 4, 128], F16, tag="png")
                                    for bi in range(4):
                                        cc = b0 + bi
                                        nc.tensor.transpose(
                                            png[:, bi, :],
                                            ng[:, cc * 128:(cc + 1) * 128], ident)
                                    nc.scalar.activation(ngT[:, b0:b0 + 4, :],
                                                         png, Act.Copy)
                                poh = pohp.tile([128, 4, 128], F32, tag="poh")
                                for mt in range(4):
                                    for k in range(8):
                                        nc.tensor.matmul(
                                            poh[:, mt, :],
                                            wo_t[:, k, mt * 128:(mt + 1) * 128],
                                            ngT[:, k, :],
                                            start=(k == 0), stop=(k == 7))
                                for mt in range(4):
                                    nc.scalar.activation(
                                        oh_all[:, mt, it * 128:(it + 1) * 128],
                                        poh[:, mt, :], Act.Silu,
                                        bias=bo_t[:, mt:mt + 1])

                    # ============ out conv + residual ============
                    with tc.tile_pool(name="s5", bufs=2) as s5p, \
                         tc.tile_pool(name="s5t", bufs=3) as s5tp:
                        og0, olen = cfg["out_g0"], cfg["out_len"]
                        hw0 = olen // 2
                        for hf in range(2):
                            o0 = og0 + hf * hw0
                            hw = hw0 if hf == 0 else olen - hw0
                            h0c = s5p.tile([128, 4, 1400], F32, tag="h0c")
                            nc.sync.dma_start(
                                out=h0c[:, :, 0:hw],
                                in_=src_r[:, :, o0 + 128:o0 + 128 + hw])
                            for m in range(4):
                                acc = s5p.tile([128, 1400], F16, tag="acc")
                                tmo = s5tp.tile([128, 1400], F16, tag="tmo")
                                nc.vector.tensor_scalar_mul(
                                    tmo[:, 0:hw],
                                    oh_all[:, m, o0 - 8:o0 - 8 + hw],
                                    ko_t[:, m, 0:1])
                                nc.vector.tensor_tensor(
                                    acc[:, 0:hw], oh_all[:, m, o0:o0 + hw],
                                    tmo[:, 0:hw], Alu.add)
                                for j in range(1, KS):
                                    tmo = s5tp.tile([128, 1400], F16, tag="tmo")
                                    nc.vector.tensor_scalar_mul(
                                        tmo[:, 0:hw],
                                        oh_all[:, m, o0 - 8 + j:o0 - 8 + j + hw],
                                        ko_t[:, m, j:j + 1])
                                    nc.vector.tensor_tensor(
                                        acc[:, 0:hw], acc[:, 0:hw], tmo[:, 0:hw],
                                        Alu.add)
                                hn = s5p.tile([128, 1400], F32, tag="hn")
                                nc.vector.tensor_tensor(hn[:, 0:hw],
                                                        h0c[:, m, 0:hw],
                                                        acc[:, 0:hw], Alu.add)
                                dst = h1d if l == 0 else h2d
                                doff = o0 - (128 if l == 0 else 256)
                                nc.sync.dma_start(
                                    out=dst[:, m, doff:doff + hw],
                                    in_=hn[:, 0:hw])
                    atp_cm.__exit__(None, None, None)

            # ================= final LN + GN + residual =================
            with tc.tile_pool(name="s6", bufs=1) as s6p, \
                 tc.tile_pool(name="s6d", bufs=2) as s6dp, \
                 tc.tile_pool(name="pf1", bufs=2, space="PSUM") as pf1, \
                 tc.tile_pool(name="pf2", bufs=2, space="PSUM") as pf2:
                h2s = s6p.tile([128, 4, TOK], F32, tag="h2s")
                nc.sync.dma_start(out=h2s, in_=h2d[:])
                mean = s6p.tile([1, TOK], F32, tag="mean")
                msq = s6p.tile([1, TOK], F32, tag="msq")
                for qq in range(4):
                    qs = qq * 512
                    ps1_ = pf1.tile([1, 512], F32, tag="ps1_")
                    ps2_ = pf1.tile([1, 512], F32, tag="ps2_")
                    for k in range(4):
                        hr = s6dp.tile([128, 512], F32R, tag="hr")
                        nc.scalar.activation(hr, h2s[:, k, qs:qs + 512], Act.Copy)
                        nc.tensor.matmul(ps1_, ones_r, hr,
                                         start=(k == 0), stop=(k == 3))
                        hq = s6dp.tile([128, 512], F32R, tag="hq")
                        nc.scalar.activation(hq, h2s[:, k, qs:qs + 512],
                                             Act.Square)
                        nc.tensor.matmul(ps2_, ones_r, hq,
                                         start=(k == 0), stop=(k == 3))
                    nc.scalar.activation(mean[:, qs:qs + 512], ps1_, Act.Copy,
                                         scale=1.0 / D)
                    nc.scalar.activation(msq[:, qs:qs + 512], ps2_, Act.Copy,
                                         scale=1.0 / D)
                var = s6p.tile([1, TOK], F32, tag="var")
                nc.vector.tensor_tensor(var, mean, mean, Alu.mult)
                nc.vector.tensor_tensor(var, msq, var, Alu.subtract)
                eps6 = s6p.tile([1, 1], F32, tag="eps6")
                nc.vector.memset(eps6, 1e-6)
                sd = s6p.tile([1, TOK], F32, tag="sd")
                nc.scalar.activation(sd, var, Act.Sqrt, bias=eps6)
                linv = s6p.tile([1, TOK], F32, tag="linv")
                nc.vector.reciprocal(linv, sd)
                mb = s6p.tile([128, TOK], F32, tag="mb")
                nc.gpsimd.partition_broadcast(mb, mean[0:1, :])
                ib = s6p.tile([128, TOK], F32, tag="ib")
                nc.gpsimd.partition_broadcast(ib, linv[0:1, :])
                hln = s6p.tile([128, 4, TOK], F32, tag="hln")
                for m in range(4):
                    nc.vector.tensor_tensor(hln[:, m, :], h2s[:, m, :], mb,
                                            Alu.subtract)
                    nc.vector.tensor_tensor(hln[:, m, :], hln[:, m, :], ib,
                                            Alu.mult)
                    nc.vector.tensor_scalar(hln[:, m, :], hln[:, m, :],
                                            lnp[:, m, 0:1], lnp[:, m, 1:2],
                                            Alu.mult, Alu.add)
                # GroupNorm stats over (D, S) per sample
                sacc = s6p.tile([128, 1], F32, tag="sacc")
                sacc2 = s6p.tile([128, 1], F32, tag="sacc2")
                for m in range(4):
                    scr = s6dp.tile([128, TOK], F16, tag="scr")
                    am = s6dp.tile([128, 1], F32, tag="am")
                    nc.scalar.activation(scr, hln[:, m, :], Act.Copy,
                                         accum_out=am)
                    scr2 = s6dp.tile([128, TOK], F16, tag="scr2")
                    am2 = s6dp.tile([128, 1], F32, tag="am2")
                    nc.scalar.activation(scr2, hln[:, m, :], Act.Square,
                                         accum_out=am2)
                    if m == 0:
                        nc.vector.tensor_copy(sacc, am)
                        nc.vector.tensor_copy(sacc2, am2)
                    else:
                        nc.vector.tensor_tensor(sacc, sacc, am, Alu.add)
                        nc.vector.tensor_tensor(sacc2, sacc2, am2, Alu.add)
                nc.sync.dma_start(out=gcc_in[0:1, 0:128], in_=sacc)
                nc.sync.dma_start(out=gcc_in[0:1, 128:256], in_=sacc2)
                nc.gpsimd.collective_compute(
                    "AllReduce", Alu.add, replica_groups=RG,
                    ins=[gcc_in.opt()], outs=[gcc_out.opt()])
                gnr = s6p.tile([1, 256], F32, tag="gnr")
                nc.sync.dma_start(out=gnr, in_=gcc_out[:])
                NTOT = float(D * S)
                gsc1 = s6p.tile([1, 128], F32, tag="gsc1")
                gsum1 = s6p.tile([1, 1], F32, tag="gsum1")
                nc.scalar.activation(gsc1, gnr[:, 0:128], Act.Copy,
                                     accum_out=gsum1)
                gsc2 = s6p.tile([1, 128], F32, tag="gsc2")
                gsum2 = s6p.tile([1, 1], F32, tag="gsum2")
                nc.scalar.activation(gsc2, gnr[:, 128:256], Act.Copy,
                                     accum_out=gsum2)
                gmean = s6p.tile([1, 1], F32, tag="gmean")
                nc.scalar.activation(gmean, gsum1, Act.Copy, scale=1.0 / NTOT)
                gm2 = s6p.tile([1, 1], F32, tag="gm2")
                nc.vector.tensor_tensor(gm2, gmean, gmean, Alu.mult)
                gvar = s6p.tile([1, 1], F32, tag="gvar")
                nc.scalar.activation(gvar, gsum2, Act.Copy, scale=1.0 / NTOT)
                nc.vector.tensor_tensor(gvar, gvar, gm2, Alu.subtract)
                eps8 = s6p.tile([1, 1], F32, tag="eps8")
                nc.vector.memset(eps8, 1e-8)
                gsd = s6p.tile([1, 1], F32, tag="gsd")
                nc.scalar.activation(gsd, gvar, Act.Sqrt, bias=eps8)
                ginv = s6p.tile([1, 1], F32, tag="ginv")
                nc.vector.reciprocal(ginv, gsd)
                gmb = s6p.tile([128, 1], F32, tag="gmb")
                nc.gpsimd.partition_broadcast(gmb, gmean[0:1, :])
                gib = s6p.tile([128, 1], F32, tag="gib")
                nc.gpsimd.partition_broadcast(gib, ginv[0:1, :])
                for m in range(4):
                    x0 = s6dp.tile([128, TOK], F32, tag="x0")
                    nc.sync.dma_start(out=x0,
                                      in_=hx_r[:, m, HXPAD:HXPAD + TOK])
                    yt = s6dp.tile([128, TOK], F32, tag="yt")
                    nc.vector.tensor_scalar(yt, hln[:, m, :], gmb, gib,
                                            Alu.subtract, Alu.mult)
                    nc.vector.tensor_scalar(yt, yt, gnp[:, m, 0:1],
                                            gnp[:, m, 1:2], Alu.mult, Alu.add)
                    nc.vector.tensor_tensor(yt, yt, x0, Alu.add)
                    nc.sync.dma_start(out=y_r[:, m, :], in_=yt)
                if DEBUG_OUT:
                    nc.sync.dma_start(out=h1dbg[:], in_=h1d[:])

    nc.compile()
    return nc


# ======================= host side =======================

def _prep_inputs(inputs):
    x = np.asarray(inputs["x"], np.float32)
    wh = np.asarray(inputs["W_hidden"], np.float32) * \
        np.asarray(inputs["g_hidden"], np.float32)[:, None, None]
    wq = np.asarray(inputs["W_qk"], np.float32) * \
        np.asarray(inputs["g_qk"], np.float32)[:, None, None]
    wo = (np.asarray(inputs["W_out"], np.float32) *
          np.asarray(inputs["g_out"], np.float32)[:, None, None]
          ).astype(np.float16)
    kh = np.asarray(inputs["k_hidden"], np.float32)
    kq = np.asarray(inputs["k_qk"], np.float32)
    ko = np.asarray(inputs["k_out"], np.float32)
    kh_t = kh.reshape(NL, 16, 128, KS).transpose(0, 2, 1, 3).copy()
    kq_t = kq.reshape(NL, 1, 128, KS).transpose(0, 2, 1, 3).copy()
    ko_t = ko.reshape(NL, 4, 128, KS).transpose(0, 2, 1, 3).copy()
    bh = np.asarray(inputs["b_hidden"], np.float32).reshape(NL, 16, 128)\
        .transpose(0, 2, 1).copy()
    bq = np.asarray(inputs["b_qk"], np.float32).reshape(NL, 1, 128)\
        .transpose(0, 2, 1).copy()
    bo = np.asarray(inputs["b_out"], np.float32).reshape(NL, 4, 128)\
        .transpose(0, 2, 1).copy()
    gam = np.asarray(inputs["gamma_qk"], np.float32).transpose(0, 2, 1).copy()
    bet = np.asarray(inputs["beta_qk"], np.float32).transpose(0, 2, 1).copy()
    lnp = np.stack([np.asarray(inputs["ln_g"], np.float32).reshape(4, 128).T,
                    np.asarray(inputs["ln_b"], np.float32).reshape(4, 128).T],
                   axis=-1).copy()
    gnp = np.stack([np.asarray(inputs["gn_w"], np.float32).reshape(4, 128).T,
                    np.asarray(inputs["gn_b"], np.float32).reshape(4, 128).T],
                   axis=-1).copy()
    P = np.zeros((32, 32), np.float32)
    for i in range(16):
        P[2 * i, 2 * i + 1] = -1.0
        P[2 * i + 1, 2 * i] = 1.0
    permT = P.T.copy()

    invf = 1.0 / (10000.0 ** (np.arange(0, 32, 2, dtype=np.float64) / 32.0))
    in_maps = []
    for c in range(8):
        b, j = c // 4, c % 4
        t0 = j * TOK
        hxb = np.zeros((D, HXLEN), np.float32)
        lo, hi = t0 - HXPAD, t0 + TOK + HXPAD
        slo, shi = max(0, lo), min(S, hi)
        hxb[:, slo - lo:shi - lo] = x[b][:, slo:shi]
        pos = np.clip(t0 - 512 + np.arange(3072, dtype=np.float64), 0, S - 1)
        fr = pos[None, :] * invf[:, None]
        fr2 = np.repeat(fr, 2, axis=0)
        in_maps.append(dict(
            hx=hxb, cosd=np.cos(fr2).astype(np.float16),
            sind=np.sin(fr2).astype(np.float16), permT=permT,
            wh_in=wh, wq_in=wq, wo_in=wo, kh_in=kh_t, kq_in=kq_t, ko_in=ko_t,
            bh_in=bh, bq_in=bq, bo_in=bo, gam_in=gam, bet_in=bet,
            ln_in=lnp, gn_in2=gnp))
    return in_maps


def _get_nc():
    if "nc" not in _CACHE:
        _CACHE["nc"] = build_nc()
    return _CACHE["nc"]


def run(inputs, trace=False, **kw):
    nc = _get_nc()
    in_maps = _prep_inputs(inputs)
    res = run_bass_kernel_spmd(nc, in_maps, list(range(8)), trace=trace, **kw)
    y = np.zeros((B, D, S), np.float32)
    for c in range(8):
        b, j = c // 4, c % 4
        y[b][:, j * TOK:(j + 1) * TOK] = res.results[c]["y_out"]
    return y, res


def kernel(**inputs) -> np.ndarray:
    y, _ = run(inputs)
    return y
